# revision 19
# baseline (speedup 1.0000x reference)
import sys
sys.path.insert(0, "/opt/trn_rl_repo")
import numpy as np
import ml_dtypes
import concourse.bass as bass
import concourse.mybir as mybir
import concourse.tile as tile
from concourse import bacc
from concourse.bass_utils import run_bass_kernel_spmd
from concourse.masks import make_identity

F32 = mybir.dt.float32
F32R = mybir.dt.float32r
BF16 = mybir.dt.bfloat16
AF = mybir.ActivationFunctionType
OP = mybir.AluOpType

S = 2048          # sequence length
H = 4096          # hidden dim
DHEAD = 128       # head dim
NQ = 4            # q heads per core
NCORES = 8
SC = 4            # s-chunks of 512
HO = 32           # h k-tiles of 128
SCALE = 1.0 / np.sqrt(128.0)
BF = ml_dtypes.bfloat16

_CACHED = {}


def _build_nc():
    nc = bacc.Bacc(None, target_bir_lowering=False, debug=False)
    # all inputs pre-swizzled host-side into SBUF partition layout, bf16
    hid_d = nc.dram_tensor("hidp", [128, SC * HO * 512], BF16, kind="ExternalInput")
    wq_d = nc.dram_tensor("wqp", [128, 6 * HO * 128], BF16, kind="ExternalInput")
    wo_d = nc.dram_tensor("wop", [128, NQ * H], BF16, kind="ExternalInput")
    cos_d = nc.dram_tensor("cos", [128, S], BF16, kind="ExternalInput")
    sin_d = nc.dram_tensor("sin", [128, S], BF16, kind="ExternalInput")
    out_d = nc.dram_tensor("out", [S, H], BF16, kind="ExternalOutput")

    with tile.TileContext(nc) as tc:
        with tc.tile_pool(name="perm", bufs=1) as perm:
            ident = perm.tile([128, 128], BF16, tag="ident")
            make_identity(nc, ident)
            ones_b = perm.tile([128, 128], BF16, tag="ones_b")
            nc.gpsimd.memset(ones_b, 1.0)
            # qT/kT/vT strips, [d=128, strip, s]: strips 0-3 = Q heads, 4 = K, 5 = V
            strips = perm.tile([128, 6, S], BF16, tag="strips")
            wq_s = perm.tile([128, 6, HO * 128], BF16, tag="wq_s")
            wo_s = perm.tile([128, NQ, H], BF16, tag="wo_s")
            attnT = perm.tile([128, NQ, S], BF16, tag="attnT")
            vnat = perm.tile([128, 16, 128], BF16, tag="vnat")
            cos_s = perm.tile([128, S], BF16, tag="cos_s")
            sin_s = perm.tile([128, S], BF16, tag="sin_s")

            # ---------------- Phase B: projections + RoPE ----------------
            # d-outer accumulation: RoPE of group d overlaps matmuls of d+1
            with tc.tile_pool(name="hc", bufs=2) as hc_p, \
                 tc.tile_pool(name="rt", bufs=2) as rt_p, \
                 tc.tile_pool(name="ppj", bufs=1, space="PSUM") as ppj:
                htcs = [hc_p.tile([128, HO, 512], BF16, tag="htc", name=f"htc{sc}")
                        for sc in range(SC)]
                for g in range(4):
                    for d in range(6):
                        nc.sync.dma_start(
                            wq_s[:, d, g * 1024:(g + 1) * 1024],
                            wq_d[:, d * HO * 128 + g * 1024:
                                 d * HO * 128 + (g + 1) * 1024])
                        if d == 0:
                            nc.sync.dma_start(
                                htcs[0][:, g * 8:g * 8 + 4, :],
                                hid_d[:, g * 4096:g * 4096 + 2048])
                        elif d == 5:
                            nc.sync.dma_start(
                                htcs[0][:, g * 8 + 4:g * 8 + 8, :],
                                hid_d[:, g * 4096 + 2048:(g + 1) * 4096])
                nc.sync.dma_start(htcs[1], hid_d[:, HO * 512:2 * HO * 512])
                nc.sync.dma_start(cos_s, cos_d[:, 0:S])
                nc.sync.dma_start(sin_s, sin_d[:, 0:S])
                for sc in range(2, SC):
                    nc.sync.dma_start(
                        htcs[sc], hid_d[:, sc * HO * 512:(sc + 1) * HO * 512])

                def rope(sc, d, psum):
                    # RoPE (strips 0-4) / copy (strip 5 = V)
                    dst = strips[:, d, sc * 512:(sc + 1) * 512]
                    cos_c = cos_s[:, sc * 512:(sc + 1) * 512]
                    sin_c = sin_s[:, sc * 512:(sc + 1) * 512]
                    if d < 5:
                        t1 = rt_p.tile([128, 512], F32, tag="t1", name="t1")
                        t2 = rt_p.tile([128, 512], F32, tag="t2", name="t2")
                        nc.vector.tensor_mul(t1, psum, cos_c)
                        nc.vector.tensor_mul(t2[0:64], psum[64:128], sin_c[0:64])
                        nc.vector.tensor_mul(t2[64:128], psum[0:64], sin_c[64:128])
                        nc.vector.tensor_add(dst, t1, t2)
                    else:
                        nc.scalar.copy(dst, psum)

                def vtrans(sc):
                    # V natural tiles for this chunk (PE transposes, trivial)
                    pt4 = ppj.tile([128, 512], BF16, tag="vtr", name="vtr")
                    for i in range(4):
                        st = 4 * sc + i
                        nc.tensor.transpose(
                            pt4[:, i * 128:(i + 1) * 128],
                            strips[:, 5, st * 128:(st + 1) * 128], ident)
                    nc.vector.tensor_copy(
                        vnat[:, 4 * sc:4 * sc + 4, :],
                        pt4.rearrange("p (a b) -> p a b", a=4))

                # chunk 0: ho-outer so PE ramps with the DMA stream
                psums0 = [ppj.tile([128, 512], F32, tag=f"pj{d}", name=f"pj{d}")
                          for d in range(6)]
                for ho in range(HO):
                    for d in range(6):
                        nc.tensor.matmul(
                            psums0[d], wq_s[:, d, ho * 128:(ho + 1) * 128],
                            htcs[0][:, ho, :],
                            start=(ho == 0), stop=(ho == HO - 1))
                for d in range(6):
                    rope(0, d, psums0[d])
                vtrans(0)
                # chunks 1-3: d-outer; RoPE of group d overlaps matmuls of d+1
                for sc in range(1, SC):
                    htc = htcs[sc]
                    for d in range(6):
                        psum = ppj.tile([128, 512], F32, tag=f"pj{d}", name=f"pj{d}")
                        for ho in range(HO):
                            nc.tensor.matmul(
                                psum, wq_s[:, d, ho * 128:(ho + 1) * 128],
                                htc[:, ho, :],
                                start=(ho == 0), stop=(ho == HO - 1))
                        rope(sc, d, psum)
                    vtrans(sc)

            # ------- Phase CD: attention (head-pair interleaved) + o_proj -------
            # den partials accumulate on DVE; one ones-matmul per (head,chunk)
            # does the partition reduction. o_proj for chunk c is interleaved
            # right after both head-pairs finish chunk c.
            for at in range(NQ):
                nc.sync.dma_start(wo_s[:, at, :], wo_d[:, at * H:(at + 1) * H])
            with tc.tile_pool(name="pmask", bufs=1) as pmask, \
                 tc.tile_pool(name="pts", bufs=8) as pts_p, \
                 tc.tile_pool(name="dac", bufs=6) as dac_p, \
                 tc.tile_pool(name="rec", bufs=4) as rec_p, \
                 tc.tile_pool(name="pos", bufs=2) as pos_p, \
                 tc.tile_pool(name="ps_s", bufs=2, space="PSUM") as s_p, \
                 tc.tile_pool(name="ps_pv", bufs=2, space="PSUM") as pv_p, \
                 tc.tile_pool(name="ps_po", bufs=2, space="PSUM") as po_p:
                ones_m = pmask.tile([128, 512], BF16, tag="ones_m")
                nc.gpsimd.memset(ones_m, 1.0)
                masks = pmask.tile([128, 4, 512], BF16, tag="masks")
                for j in range(4):
                    nc.gpsimd.affine_select(
                        out=masks[:, j, :], in_=ones_m, pattern=[[1, 512]],
                        compare_op=OP.is_ge, fill=0.0,
                        base=-128 * j, channel_multiplier=-1)
                for c in range(SC):
                    nkt = 4 * c + 4
                    for hp in range(2):
                        heads = (2 * hp, 2 * hp + 1)
                        pvs = [pv_p.tile([128, 512], F32, tag="pv", name="pv")
                               for _ in range(2)]
                        accs = [dac_p.tile([128, 512], BF16, tag="ac", name="ac")
                                for _ in range(2)]
                        for p in range(nkt // 2):
                            ptiles = []
                            for i, h in enumerate(heads):
                                sp = s_p.tile([128, 2, 512], F32, tag="s")
                                for j in range(2):
                                    kt = 2 * p + j
                                    nc.tensor.matmul(
                                        sp[:, j, :],
                                        strips[:, 4, kt * 128:(kt + 1) * 128],
                                        strips[:, h, c * 512:(c + 1) * 512],
                                        start=True, stop=True)
                                ptile = pts_p.tile([128, 2, 512], BF16, tag="pt")
                                nc.scalar.activation(ptile, sp, AF.Exp, scale=SCALE)
                                jm = 2 * p - 4 * c
                                if jm >= 0:
                                    nc.vector.tensor_mul(
                                        ptile, ptile, masks[:, jm:jm + 2, :])
                                ptiles.append(ptile)
                            for i in range(2):
                                for j in range(2):
                                    kt = 2 * p + j
                                    nc.tensor.matmul(
                                        pvs[i], vnat[:, kt, :], ptiles[i][:, j, :],
                                        start=(kt == 0), stop=(kt == nkt - 1))
                                if p == 0:
                                    nc.vector.tensor_add(
                                        accs[i], ptiles[i][:, 0, :], ptiles[i][:, 1, :])
                                else:
                                    nc.vector.tensor_add(
                                        accs[i], accs[i], ptiles[i][:, 0, :])
                                    nc.vector.tensor_add(
                                        accs[i], accs[i], ptiles[i][:, 1, :])
                        for i, h in enumerate(heads):
                            # partition-reduce acc -> den via one PE matmul
                            dps = po_p.tile([128, 512], F32, tag="po", name="dps")
                            nc.tensor.matmul(dps, ones_b, accs[i],
                                             start=True, stop=True)
                            rec = rec_p.tile([128, 512], F32, tag="rec")
                            nc.vector.reciprocal_approx_fast(rec, dps)
                            nc.vector.tensor_mul(
                                attnT[:, h, c * 512:(c + 1) * 512], pvs[i], rec)
                    # o_proj for this chunk's 4 s-tiles
                    for st in range(4 * c, 4 * c + 4):
                        pos = pos_p.tile([128, H], BF16, tag="pos")
                        for mc in range(8):
                            po = po_p.tile([128, 512], F32, tag="po")
                            for at in range(NQ):
                                nc.tensor.matmul(
                                    po, attnT[:, at, st * 128:(st + 1) * 128],
                                    wo_s[:, at, mc * 512:(mc + 1) * 512],
                                    start=(at == 0), stop=(at == 3))
                            if mc % 2 == 0:
                                nc.scalar.copy(pos[:, mc * 512:(mc + 1) * 512], po)
                            else:
                                nc.vector.tensor_copy(
                                    pos[:, mc * 512:(mc + 1) * 512], po)
                        if st == 15:
                            for g in range(4):
                                nc.sync.dma_start(
                                    out_d[st * 128:(st + 1) * 128,
                                          g * 1024:(g + 1) * 1024],
                                    pos[:, g * 1024:(g + 1) * 1024])
                        else:
                            nc.sync.dma_start(out_d[st * 128:(st + 1) * 128, :], pos)
    nc.compile()
    return nc


def _prepare(hidden_states, position_ids, Wq, Wk, Wv, Wo):
    hidden_states = np.asarray(hidden_states, dtype=np.float32)
    position_ids = np.asarray(position_ids)
    Wq = np.asarray(Wq, dtype=np.float32)
    Wk = np.asarray(Wk, dtype=np.float32)
    Wv = np.asarray(Wv, dtype=np.float32)
    Wo = np.asarray(Wo, dtype=np.float32)

    # RoPE tables in [d=128, s] layout; sin has sign folded for rotate_half
    pos = position_ids.reshape(-1).astype(np.float64)  # [S]
    invf = 1.0 / (10000.0 ** (np.arange(0, 128, 2, dtype=np.float64) / 128.0))
    ang = invf[:, None] * pos[None, :]                 # [64, S]
    cos_t = np.ascontiguousarray(np.concatenate([np.cos(ang), np.cos(ang)], axis=0)).astype(BF)
    sin_t = np.ascontiguousarray(np.concatenate([-np.sin(ang), np.sin(ang)], axis=0)).astype(BF)

    # hidden pack: [p, (sc*HO + ho)*512 + j] = hidden[sc*512+j, ho*128+p]
    hidT = hidden_states[0].T.astype(BF)                       # [H, S]
    hid_pack = np.ascontiguousarray(
        hidT.reshape(HO, 128, SC, 512).transpose(1, 2, 0, 3).reshape(128, SC * HO * 512))

    WoT = np.ascontiguousarray(Wo.T)                           # [H, H]
    in_maps = []
    for c in range(NCORES):
        wqkvT = np.concatenate([
            Wq[c * 512:(c + 1) * 512],
            Wk[c * 128:(c + 1) * 128],
            Wv[c * 128:(c + 1) * 128]], axis=0).T.astype(BF)   # [H, 768]
        # wq pack: [p, (d*HO + ho)*128 + j] = wqkvT[ho*128+p, d*128+j]
        wq_pack = np.ascontiguousarray(
            wqkvT.reshape(HO, 128, 6, 128).transpose(1, 2, 0, 3).reshape(128, 6 * HO * 128))
        # wo pack: [p, at*H + col] = WoT[c*512 + at*128 + p, col]
        woT_c = WoT[c * 512:(c + 1) * 512].astype(BF)          # [512, H]
        wo_pack = np.ascontiguousarray(
            woT_c.reshape(NQ, 128, H).transpose(1, 0, 2).reshape(128, NQ * H))
        in_maps.append({"hidp": hid_pack, "wqp": wq_pack, "wop": wo_pack,
                        "cos": cos_t, "sin": sin_t})
    return in_maps


def kernel(hidden_states, position_ids, Wq, Wk, Wv, Wo, **extra):
    B = np.asarray(hidden_states).shape[0]
    assert B == 1

    if "nc" not in _CACHED:
        _CACHED["nc"] = _build_nc()
    nc = _CACHED["nc"]

    in_maps = _prepare(hidden_states, position_ids, Wq, Wk, Wv, Wo)
    res = run_bass_kernel_spmd(nc, in_maps, core_ids=list(range(NCORES)))
    out = np.zeros((S, H), dtype=np.float32)
    for c in range(NCORES):
        out += res.results[c]["out"].astype(np.float32)
    return out.reshape(1, S, H)


# revision 20
# speedup vs baseline: 1.0421x; 1.0421x over previous
import sys
sys.path.insert(0, "/opt/trn_rl_repo")
import numpy as np
import ml_dtypes
import concourse.bass as bass
import concourse.mybir as mybir
import concourse.tile as tile
from concourse import bacc
from concourse.bass_utils import run_bass_kernel_spmd
from concourse.masks import make_identity

F32 = mybir.dt.float32
F32R = mybir.dt.float32r
BF16 = mybir.dt.bfloat16
AF = mybir.ActivationFunctionType
OP = mybir.AluOpType

S = 2048          # sequence length
H = 4096          # hidden dim
DHEAD = 128       # head dim
NQ = 4            # q heads per core
NCORES = 8
SC = 4            # s-chunks of 512
HO = 32           # h k-tiles of 128
SCALE = 1.0 / np.sqrt(128.0)
BF = ml_dtypes.bfloat16

_CACHED = {}


def _build_nc():
    nc = bacc.Bacc(None, target_bir_lowering=False, debug=False)
    # all inputs pre-swizzled host-side into SBUF partition layout, bf16
    hid_d = nc.dram_tensor("hidp", [128, SC * HO * 512], BF16, kind="ExternalInput")
    wq_d = nc.dram_tensor("wqp", [128, 6 * HO * 128], BF16, kind="ExternalInput")
    wo_d = nc.dram_tensor("wop", [128, NQ * H], BF16, kind="ExternalInput")
    cos_d = nc.dram_tensor("cos", [128, S], BF16, kind="ExternalInput")
    sin_d = nc.dram_tensor("sin", [128, S], BF16, kind="ExternalInput")
    out_d = nc.dram_tensor("out", [S, H], BF16, kind="ExternalOutput")

    with tile.TileContext(nc) as tc:
        with tc.tile_pool(name="perm", bufs=1) as perm:
            ident = perm.tile([128, 128], BF16, tag="ident")
            make_identity(nc, ident)
            ones_b = perm.tile([128, 128], BF16, tag="ones_b")
            nc.gpsimd.memset(ones_b, 1.0)
            # qT/kT/vT strips, [d=128, strip, s]: strips 0-3 = Q heads, 4 = K, 5 = V
            strips = perm.tile([128, 6, S], BF16, tag="strips")
            wq_s = perm.tile([128, 6, HO * 128], BF16, tag="wq_s")
            wo_s = perm.tile([128, NQ, H], BF16, tag="wo_s")
            attnT = perm.tile([128, NQ, S], BF16, tag="attnT")
            vnat = perm.tile([128, 16, 128], BF16, tag="vnat")
            cos_s = perm.tile([128, S], BF16, tag="cos_s")
            sin_s = perm.tile([128, S], BF16, tag="sin_s")

            # ---------------- Phase B: projections + RoPE ----------------
            # d-outer accumulation: RoPE of group d overlaps matmuls of d+1
            with tc.tile_pool(name="hc", bufs=2) as hc_p, \
                 tc.tile_pool(name="rt", bufs=2) as rt_p, \
                 tc.tile_pool(name="ppj", bufs=1, space="PSUM") as ppj:
                htcs = [hc_p.tile([128, HO, 512], BF16, tag="htc", name=f"htc{sc}")
                        for sc in range(SC)]
                for g in range(4):
                    for d in range(6):
                        nc.sync.dma_start(
                            wq_s[:, d, g * 1024:(g + 1) * 1024],
                            wq_d[:, d * HO * 128 + g * 1024:
                                 d * HO * 128 + (g + 1) * 1024])
                        if d == 0:
                            nc.sync.dma_start(
                                htcs[0][:, g * 8:g * 8 + 4, :],
                                hid_d[:, g * 4096:g * 4096 + 2048])
                        elif d == 5:
                            nc.sync.dma_start(
                                htcs[0][:, g * 8 + 4:g * 8 + 8, :],
                                hid_d[:, g * 4096 + 2048:(g + 1) * 4096])
                nc.sync.dma_start(htcs[1], hid_d[:, HO * 512:2 * HO * 512])
                nc.sync.dma_start(cos_s, cos_d[:, 0:S])
                nc.sync.dma_start(sin_s, sin_d[:, 0:S])
                for sc in range(2, SC):
                    nc.sync.dma_start(
                        htcs[sc], hid_d[:, sc * HO * 512:(sc + 1) * HO * 512])

                def rope(sc, d, psum):
                    # RoPE (strips 0-4) / copy (strip 5 = V)
                    dst = strips[:, d, sc * 512:(sc + 1) * 512]
                    cos_c = cos_s[:, sc * 512:(sc + 1) * 512]
                    sin_c = sin_s[:, sc * 512:(sc + 1) * 512]
                    if d < 5:
                        t1 = rt_p.tile([128, 512], F32, tag="t1", name="t1")
                        t2 = rt_p.tile([128, 512], F32, tag="t2", name="t2")
                        nc.vector.tensor_mul(t1, psum, cos_c)
                        nc.vector.tensor_mul(t2[0:64], psum[64:128], sin_c[0:64])
                        nc.vector.tensor_mul(t2[64:128], psum[0:64], sin_c[64:128])
                        nc.vector.tensor_add(dst, t1, t2)
                    else:
                        nc.scalar.copy(dst, psum)

                def vtrans(sc):
                    # V natural tiles for this chunk (PE transposes, trivial)
                    pt4 = ppj.tile([128, 512], BF16, tag="vtr", name="vtr")
                    for i in range(4):
                        st = 4 * sc + i
                        nc.tensor.transpose(
                            pt4[:, i * 128:(i + 1) * 128],
                            strips[:, 5, st * 128:(st + 1) * 128], ident)
                    nc.vector.tensor_copy(
                        vnat[:, 4 * sc:4 * sc + 4, :],
                        pt4.rearrange("p (a b) -> p a b", a=4))

                # chunk 0: ho-outer so PE ramps with the DMA stream
                psums0 = [ppj.tile([128, 512], F32, tag=f"pj{d}", name=f"pj{d}")
                          for d in range(6)]
                for ho in range(HO):
                    for d in range(6):
                        nc.tensor.matmul(
                            psums0[d], wq_s[:, d, ho * 128:(ho + 1) * 128],
                            htcs[0][:, ho, :],
                            start=(ho == 0), stop=(ho == HO - 1))
                for d in range(6):
                    rope(0, d, psums0[d])
                vtrans(0)
                # chunks 1-3: d-outer; RoPE of group d overlaps matmuls of d+1
                for sc in range(1, SC):
                    htc = htcs[sc]
                    for d in range(6):
                        psum = ppj.tile([128, 512], F32, tag=f"pj{d}", name=f"pj{d}")
                        for ho in range(HO):
                            nc.tensor.matmul(
                                psum, wq_s[:, d, ho * 128:(ho + 1) * 128],
                                htc[:, ho, :],
                                start=(ho == 0), stop=(ho == HO - 1))
                        rope(sc, d, psum)
                    vtrans(sc)

            # ------- Phase CD: attention (head-pair interleaved) + o_proj -------
            # den partials accumulate on DVE; one ones-matmul per (head,chunk)
            # does the partition reduction. o_proj for chunk c is interleaved
            # right after both head-pairs finish chunk c.
            for at in range(NQ):
                nc.sync.dma_start(wo_s[:, at, :], wo_d[:, at * H:(at + 1) * H])
            with tc.tile_pool(name="pmask", bufs=1) as pmask, \
                 tc.tile_pool(name="pts", bufs=8) as pts_p, \
                 tc.tile_pool(name="dac", bufs=6) as dac_p, \
                 tc.tile_pool(name="rec", bufs=4) as rec_p, \
                 tc.tile_pool(name="pos", bufs=2) as pos_p, \
                 tc.tile_pool(name="ps_s", bufs=2, space="PSUM") as s_p, \
                 tc.tile_pool(name="ps_pv", bufs=2, space="PSUM") as pv_p, \
                 tc.tile_pool(name="ps_po", bufs=2, space="PSUM") as po_p:
                ones_m = pmask.tile([128, 512], BF16, tag="ones_m")
                nc.gpsimd.memset(ones_m, 1.0)
                masks = pmask.tile([128, 4, 512], BF16, tag="masks")
                for j in range(4):
                    nc.gpsimd.affine_select(
                        out=masks[:, j, :], in_=ones_m, pattern=[[1, 512]],
                        compare_op=OP.is_ge, fill=0.0,
                        base=-128 * j, channel_multiplier=-1)
                for c in range(SC):
                    nkt = 4 * c + 4
                    for hp in range(2):
                        heads = (2 * hp, 2 * hp + 1)
                        pvs = [pv_p.tile([128, 512], F32, tag="pv", name="pv")
                               for _ in range(2)]
                        accs = [dac_p.tile([128, 512], BF16, tag="ac", name="ac")
                                for _ in range(2)]
                        for p in range(nkt // 2):
                            jm = 2 * p - 4 * c
                            # second diagonal pair: columns [0:256) are fully
                            # masked -> compute only the upper 256 columns
                            off = 256 if jm == 2 else 0
                            ptiles = []
                            for i, h in enumerate(heads):
                                sp = s_p.tile([128, 2, 512], F32, tag="s")
                                for j in range(2):
                                    kt = 2 * p + j
                                    nc.tensor.matmul(
                                        sp[:, j, off:512],
                                        strips[:, 4, kt * 128:(kt + 1) * 128],
                                        strips[:, h, c * 512 + off:(c + 1) * 512],
                                        start=True, stop=True)
                                ptile = pts_p.tile([128, 2, 512], BF16, tag="pt")
                                nc.scalar.activation(
                                    ptile[:, :, off:512], sp[:, :, off:512],
                                    AF.Exp, scale=SCALE)
                                if jm >= 0:
                                    nc.vector.tensor_mul(
                                        ptile[:, :, off:512], ptile[:, :, off:512],
                                        masks[:, jm:jm + 2, off:512])
                                ptiles.append(ptile)
                            for i in range(2):
                                for j in range(2):
                                    kt = 2 * p + j
                                    nc.tensor.matmul(
                                        pvs[i][:, off:512], vnat[:, kt, :],
                                        ptiles[i][:, j, off:512],
                                        start=(kt == 0), stop=(kt == nkt - 1),
                                        skip_group_check=True)
                                if p == 0:
                                    nc.vector.tensor_add(
                                        accs[i], ptiles[i][:, 0, :], ptiles[i][:, 1, :])
                                else:
                                    nc.vector.tensor_add(
                                        accs[i][:, off:512], accs[i][:, off:512],
                                        ptiles[i][:, 0, off:512])
                                    nc.vector.tensor_add(
                                        accs[i][:, off:512], accs[i][:, off:512],
                                        ptiles[i][:, 1, off:512])
                        for i, h in enumerate(heads):
                            # partition-reduce acc -> den via one PE matmul
                            dps = po_p.tile([128, 512], F32, tag="po", name="dps")
                            nc.tensor.matmul(dps, ones_b, accs[i],
                                             start=True, stop=True)
                            rec = rec_p.tile([128, 512], F32, tag="rec")
                            nc.vector.reciprocal_approx_fast(rec, dps)
                            nc.vector.tensor_mul(
                                attnT[:, h, c * 512:(c + 1) * 512], pvs[i], rec)
                    # o_proj for this chunk's 4 s-tiles
                    for st in range(4 * c, 4 * c + 4):
                        pos = pos_p.tile([128, H], BF16, tag="pos")
                        for mc in range(8):
                            po = po_p.tile([128, 512], F32, tag="po")
                            for at in range(NQ):
                                nc.tensor.matmul(
                                    po, attnT[:, at, st * 128:(st + 1) * 128],
                                    wo_s[:, at, mc * 512:(mc + 1) * 512],
                                    start=(at == 0), stop=(at == 3))
                            if mc % 2 == 0:
                                nc.scalar.copy(pos[:, mc * 512:(mc + 1) * 512], po)
                            else:
                                nc.vector.tensor_copy(
                                    pos[:, mc * 512:(mc + 1) * 512], po)
                        if st == 15:
                            for g in range(4):
                                nc.sync.dma_start(
                                    out_d[st * 128:(st + 1) * 128,
                                          g * 1024:(g + 1) * 1024],
                                    pos[:, g * 1024:(g + 1) * 1024])
                        else:
                            nc.sync.dma_start(out_d[st * 128:(st + 1) * 128, :], pos)
    nc.compile()
    return nc


def _prepare(hidden_states, position_ids, Wq, Wk, Wv, Wo):
    hidden_states = np.asarray(hidden_states, dtype=np.float32)
    position_ids = np.asarray(position_ids)
    Wq = np.asarray(Wq, dtype=np.float32)
    Wk = np.asarray(Wk, dtype=np.float32)
    Wv = np.asarray(Wv, dtype=np.float32)
    Wo = np.asarray(Wo, dtype=np.float32)

    # RoPE tables in [d=128, s] layout; sin has sign folded for rotate_half
    pos = position_ids.reshape(-1).astype(np.float64)  # [S]
    invf = 1.0 / (10000.0 ** (np.arange(0, 128, 2, dtype=np.float64) / 128.0))
    ang = invf[:, None] * pos[None, :]                 # [64, S]
    cos_t = np.ascontiguousarray(np.concatenate([np.cos(ang), np.cos(ang)], axis=0)).astype(BF)
    sin_t = np.ascontiguousarray(np.concatenate([-np.sin(ang), np.sin(ang)], axis=0)).astype(BF)

    # hidden pack: [p, (sc*HO + ho)*512 + j] = hidden[sc*512+j, ho*128+p]
    hidT = hidden_states[0].T.astype(BF)                       # [H, S]
    hid_pack = np.ascontiguousarray(
        hidT.reshape(HO, 128, SC, 512).transpose(1, 2, 0, 3).reshape(128, SC * HO * 512))

    WoT = np.ascontiguousarray(Wo.T)                           # [H, H]
    in_maps = []
    for c in range(NCORES):
        wqkvT = np.concatenate([
            Wq[c * 512:(c + 1) * 512],
            Wk[c * 128:(c + 1) * 128],
            Wv[c * 128:(c + 1) * 128]], axis=0).T.astype(BF)   # [H, 768]
        # wq pack: [p, (d*HO + ho)*128 + j] = wqkvT[ho*128+p, d*128+j]
        wq_pack = np.ascontiguousarray(
            wqkvT.reshape(HO, 128, 6, 128).transpose(1, 2, 0, 3).reshape(128, 6 * HO * 128))
        # wo pack: [p, at*H + col] = WoT[c*512 + at*128 + p, col]
        woT_c = WoT[c * 512:(c + 1) * 512].astype(BF)          # [512, H]
        wo_pack = np.ascontiguousarray(
            woT_c.reshape(NQ, 128, H).transpose(1, 0, 2).reshape(128, NQ * H))
        in_maps.append({"hidp": hid_pack, "wqp": wq_pack, "wop": wo_pack,
                        "cos": cos_t, "sin": sin_t})
    return in_maps


def kernel(hidden_states, position_ids, Wq, Wk, Wv, Wo, **extra):
    B = np.asarray(hidden_states).shape[0]
    assert B == 1

    if "nc" not in _CACHED:
        _CACHED["nc"] = _build_nc()
    nc = _CACHED["nc"]

    in_maps = _prepare(hidden_states, position_ids, Wq, Wk, Wv, Wo)
    res = run_bass_kernel_spmd(nc, in_maps, core_ids=list(range(NCORES)))
    out = np.zeros((S, H), dtype=np.float32)
    for c in range(NCORES):
        out += res.results[c]["out"].astype(np.float32)
    return out.reshape(1, S, H)
